# revision 29
# baseline (speedup 1.0000x reference)
"""Trainium2 Bass kernel for the CNF (continuous normalizing flow) problem.

Computes, for x ~ (16384, 8): 8 fixed dopri5 steps of the augmented ODE
  dx/dt = f(t, x) = tanh(tanh([x,t]W1+b1)W2+b2)W3 + b3
  dlogdet/dt = -tr(df/dx)
returning (z, delta_log_det).

Key reductions (host-precomputed constants):
  * tr(J) = d1^T C d2 with d_i = 1-h_i^2 and C[b,a] = W2[b,a]*(W3@W1[:8])[a,b].
    With s_i = h_i^2 and P = s1@C:  tr = K0 - c1.s2 - sum_a P_a(1-s2_a),
    needing only two squares, one extra matmul, and one fused (s2-1)*P op;
    all feature reductions ride inside matmul columns.
  * dopri5 stage combines x + dt*sum_j a_sj k_j fold into MM1's contraction:
    k_j tiles stack along partitions at 32-aligned offsets and MM1 adds one
    stacked pass with pre-scaled W1' blocks.  t/b3 feed-through terms fold
    into the per-eval tanh bias table.
  * the logdet pieces (m = c1.s2 - sum w2t per stage) accumulate directly in
    the y-update PSUM bank across the step with dt*b_j pre-scaled weights, so
    they never touch the critical path; stage 2's trace branch is skipped
    entirely (b[1] = 0).
  * state matmul passes run as float32r (1 cycle/row vs 4 for plain fp32).

Sharding: pure batch data-parallel over 8 cores (2048 samples/core).
Per-core layout: 4 chunks of 512 samples as 2 "pairs"; feature dim on
partitions (64 features x 2 chunks = 128 partitions).
"""

import os

import numpy as np

D = 8
H = 64
STEPS = 8
BATCH = 16384
NCORES = 8
NPC = BATCH // NCORES          # samples per core
WCH = 512                      # chunk width (columns per matmul stream)
NCHUNK = NPC // WCH            # 4
NPAIR = NCHUNK // 2            # 2
KR = 2 * D                     # 16 kx rows per pair
SR = 34                        # state tile rows: x at 0:16, ld at 32:34

DT = 1.0 / STEPS

# dopri5 tableau (matches reference._dopri5_step)
_A = [
    [],
    [0.2],
    [3 / 40, 9 / 40],
    [44 / 45, -56 / 15, 32 / 9],
    [19372 / 6561, -25360 / 2187, 64448 / 6561, -212 / 729],
    [9017 / 3168, -355 / 33, 46732 / 5247, 49 / 176, -5103 / 18656],
]
_B = [35 / 384, 0.0, 500 / 1113, 125 / 192, -2187 / 6784, 11 / 84]
_C = [0.0, 0.2, 0.3, 0.8, 8 / 9, 1.0]
NEVAL = 6 * STEPS

EVAL_BF16 = os.environ.get("CNF_FP32", "0") != "1"


def _host_consts(W1, b1, W2, b2, W3, b3):
    """All matmul weight blocks / bias tables, as fp32 numpy arrays."""
    f32 = np.float32
    W1 = np.asarray(W1, f32); b1 = np.asarray(b1, f32)
    W2 = np.asarray(W2, f32); b2 = np.asarray(b2, f32)
    W3 = np.asarray(W3, f32); b3 = np.asarray(b3, f32)
    W1p = W1[:D]                      # (8, 64)
    W1t = W1[D]                       # (64,)
    M = W3 @ W1p                      # (64, 64)
    C = W2 * M.T                      # C[b,a] = W2[b,a] * M[a,b]
    c1 = C.sum(axis=0)                # (64,)
    K0 = float(C.sum())
    b3W1 = b3 @ W1p                   # (64,)

    # per-eval tanh-1 bias: b1 + t*W1[8,:] + dt*(n + C_s)*(b3@W1')
    bias1 = np.zeros((128, NEVAL), f32)
    for n in range(STEPS):
        for s in range(6):
            e = 6 * n + s
            te = (n + _C[s]) * DT
            v = b1 + te * W1t + te * b3W1
            bias1[:H, e] = v
            bias1[H:, e] = v
    bias2 = np.concatenate([b2, b2]).reshape(128, 1).astype(f32)

    def blk(coef):
        """16-row block of scaled W1' for one k_j group of an MM1 lhsT."""
        b = np.zeros((32, 128), f32)
        b[0:D, 0:H] = coef * W1p
        b[D:2 * D, H:2 * H] = coef * W1p
        return b

    w1s = np.zeros((SR, 128), f32)    # state pass lhsT (34, 128)
    w1s[:KR] = blk(1.0)[:KR]

    # k_j groups: j=0..3 in kxgA at rows 32j, j=4 in kxgB at row 0
    def kstack(coefs, nrows):
        m = np.zeros((nrows, 128), f32)
        for j, cf in coefs:
            m[32 * j:32 * j + KR] = blk(cf)[:KR]
        return m

    # h2-direct: z1 contribution of k_j rides h2_j @ (coef * W31) with
    # W31 = W3 @ W1p, so k tiles are never materialized.
    W31 = W3 @ W1p                    # (64, 64)

    def w31blk(coef):
        b = np.zeros((128, 128), f32)
        b[0:H, 0:H] = coef * W31
        b[H:, H:] = coef * W31
        return b

    w31g = {}                         # (s, j) -> lhsT for stage s, k_j
    for s in range(1, 6):
        for j in range(s):
            w31g[(s, j)] = w31blk(DT * _A[s][j])

    w2blk = np.zeros((128, 128), f32)
    w2blk[0:H, 0:H] = W2
    w2blk[H:, H:] = W2
    cblk = np.zeros((128, 128), f32)
    cblk[0:H, 0:H] = C
    cblk[H:, H:] = C


    # m-accumulators, pre-scaled by dt*b_s, writing yn rows 16:18
    m3b = []
    m3c = []
    for s in range(6):
        bpos = np.zeros((128, 2), f32)
        bpos[0:H, 0] = DT * _B[s] * c1
        bpos[H:, 1] = DT * _B[s] * c1
        m3b.append(bpos)
        bneg = np.zeros((128, 2), f32)
        bneg[0:H, 0] = -DT * _B[s]
        bneg[H:, 1] = -DT * _B[s]
        m3c.append(bneg)

    yupd_s = np.zeros((SR, SR), f32)  # carries x (0:16) and ld (32:34)
    for r in list(range(KR)) + [32, 33]:
        yupd_s[r, r] = 1.0

    w3y = {}                          # j -> y-update pass lhsT over h2_j
    for j in range(6):
        if _B[j] == 0.0:
            continue
        m = np.zeros((128, SR), f32)
        m[0:H, 0:D] = DT * _B[j] * W3
        m[H:, D:2 * D] = DT * _B[j] * W3
        w3y[j] = m

    return dict(w1s=w1s, w31g=w31g, w2blk=w2blk,
                cblk=cblk, m3b=m3b, m3c=m3c, yupd_s=yupd_s, w3y=w3y,
                bias1=bias1, bias2=bias2, K0=K0, b3=b3)


def _build(consts):
    """Build + compile the Bass module."""
    import ml_dtypes
    import concourse.bacc as bacc
    import concourse.mybir as mybir
    from concourse.tile import TileContext

    f32 = mybir.dt.float32
    f32r = mybir.dt.float32r
    edt = mybir.dt.bfloat16 if EVAL_BF16 else mybir.dt.float32
    enp = ml_dtypes.bfloat16 if EVAL_BF16 else np.float32

    nc = bacc.Bacc("TRN2", target_bir_lowering=False, debug=False)

    # x/z are host-transposed to feature-major for contiguous DMA
    x_d = nc.dram_tensor("x", [D, NPC], f32, kind="ExternalInput")
    z_d = nc.dram_tensor("z", [D, NPC], f32, kind="ExternalOutput")
    ld_d = nc.dram_tensor("ld", [NCHUNK, WCH], f32, kind="ExternalOutput")

    def inl(name, arr, npdt):
        return nc.inline_tensor(np.ascontiguousarray(arr.astype(npdt)),
                                name=name)

    # pack all lhsT/bias constants into two DRAM blocks (one DMA each)
    epack = []          # (name, array) -> column offsets, bf16 block
    fpack = []          # fp32 block
    def eadd(name, arr):
        a = np.zeros((128, arr.shape[1]), np.float32)
        a[:arr.shape[0]] = arr
        epack.append((name, a))
    def fadd(name, arr):
        a = np.zeros((128, arr.shape[1]), np.float32)
        a[:arr.shape[0]] = arr
        fpack.append((name, a))
    for k, v in consts["w31g"].items():
        eadd(f"w31g{k[0]}_{k[1]}", v)
    eadd("w2blk", consts["w2blk"])
    eadd("cblk", consts["cblk"])
    for s in range(6):
        eadd(f"m3b{s}", consts["m3b"][s])
        eadd(f"m3c{s}", consts["m3c"][s])
    for j, v in consts["w3y"].items():
        eadd(f"w3y{j}", v)
    fadd("w1s", consts["w1s"])
    fadd("yus", consts["yupd_s"])
    fadd("b1t", consts["bias1"])
    fadd("b2d", consts["bias2"])
    eoff = {}
    off = 0
    for name, a in epack:
        eoff[name] = (off, a.shape[1])
        off += a.shape[1]
    eblob = np.concatenate([a for _, a in epack], axis=1)
    foff = {}
    off = 0
    for name, a in fpack:
        foff[name] = (off, a.shape[1])
        off += a.shape[1]
    fblob = np.concatenate([a for _, a in fpack], axis=1)
    d_eblob = inl("eblob", eblob, enp)
    d_fblob = inl("fblob", fblob, np.float32)

    Tanh = mybir.ActivationFunctionType.Tanh
    Square = mybir.ActivationFunctionType.Square
    sub = mybir.AluOpType.subtract
    mul = mybir.AluOpType.mult

    with TileContext(nc) as tc:
        with tc.tile_pool(name="const", bufs=1) as cp, \
             tc.tile_pool(name="state", bufs=2) as sp, \
             tc.tile_pool(name="kxg", bufs=1) as kp, \
             tc.tile_pool(name="work", bufs=3) as wp, \
             tc.tile_pool(name="ps", bufs=2, space="PSUM") as pp, \
             tc.tile_pool(name="psg", bufs=1, space="PSUM") as pg:

            # state tiles first so their DMAs lead the queue.
            # state is f32r (1 cycle/row matmuls); producers must round.
            ystate = []
            xa = x_d.ap()
            for p in range(NPAIR):
                xs = sp.tile([SR, WCH], f32, tag=f"xs{p}")
                nc.vector.memset(xs[:], 0.0)
                for q in range(2):
                    c = 2 * p + q
                    nc.sync.dma_start(
                        out=xs[q * D:(q + 1) * D, :],
                        in_=xa[:, c * WCH:(c + 1) * WCH])
                st = sp.tile([SR, WCH], f32r, tag=f"y{p}")
                nc.vector.tensor_copy(out=st[:], in_=xs[:])
                ystate.append(st)

            ce = cp.tile([128, sum(w.shape[1] for _, w in epack)], edt,
                         name="c_eblob")
            nc.sync.dma_start(out=ce[:], in_=d_eblob.ap())
            cf = cp.tile([128, sum(w.shape[1] for _, w in fpack)], f32,
                         name="c_fblob")
            nc.sync.dma_start(out=cf[:], in_=d_fblob.ap())

            def esl(name, rows=128):
                o, w = eoff[name]
                return ce[0:rows, o:o + w]
            def fsl(name, rows=128):
                o, w = foff[name]
                return cf[0:rows, o:o + w]

            c_w31g = {k: esl(f"w31g{k[0]}_{k[1]}")
                      for k in consts["w31g"]}
            c_w2blk = esl("w2blk")
            c_cblk = esl("cblk")
            c_m3b = [esl(f"m3b{s}") for s in range(6)]
            c_m3c = [esl(f"m3c{s}") for s in range(6)]
            c_w3y = {j: esl(f"w3y{j}") for j in consts["w3y"]}
            crs = cp.tile([SR, 2 * SR + 128], f32r, name="c_rblob")
            nc.vector.tensor_copy(
                out=crs[:, 0:128], in_=fsl("w1s", SR))
            nc.vector.tensor_copy(
                out=crs[:, 128:128 + SR], in_=fsl("yus", SR))
            c_w1s = crs[:, 0:128]
            c_yus = crs[:, 128:128 + SR]
            c_b1t = fsl("b1t")
            c_b2 = fsl("b2d")

            P = range(NPAIR)
            for n in range(STEPS):
                # y-update accumulator: state carried over via identity
                # (start=True clears the bank), per-stage m-terms accumulate
                # into rows 32:34, dt*b_j k-combination lands at step end
                yn = [pg.tile([SR, WCH], f32, tag=f"yn{p}", name=f"yn{p}_{n}")
                      for p in P]
                for p in P:
                    nc.tensor.matmul(yn[p][:], c_yus,
                                     ystate[p][:],
                                     start=True, stop=False)
                pending_m = []      # deferred yn-accum matmuls
                h2s = {}            # stage index -> per-pair h2 tiles
                for s in range(6):
                    e = 6 * n + s
                    trace_on = _B[s] != 0.0
                    # -- critical chain: z1 -> tanh1 -> z2 -> tanh2 -> kx --
                    z1 = [pp.tile([128, WCH], f32, tag="zz", bufs=4,
                                  name=f"z1_{p}_{e}") for p in P]
                    for p in P:
                        nc.tensor.matmul(z1[p][:], c_w1s,
                                         ystate[p][:],
                                         start=True, stop=(s == 0))
                    for j in range(s):
                        for p in P:
                            nc.tensor.matmul(
                                z1[p][:], c_w31g[(s, j)], h2s[j][p][:],
                                start=False, stop=(j == s - 1))
                    h1 = [wp.tile([128, WCH], edt, tag="h1",
                                  name=f"h1_{p}_{e}") for p in P]
                    for p in P:
                        nc.scalar.activation(h1[p][:], z1[p][:], Tanh,
                                             bias=c_b1t[:, e:e + 1])
                    z2 = [pp.tile([128, WCH], f32, tag="zz", bufs=4,
                                  name=f"z2_{p}_{e}") for p in P]
                    for p in P:
                        nc.tensor.matmul(z2[p][:], c_w2blk, h1[p][:],
                                         start=True, stop=True)
                    # deferred yn-accums from the previous stage fill PE's
                    # wait for tanh2
                    for lhsT, rhs, pp_, mode in pending_m:
                        dst = yn[pp_][:] if mode == "full" else yn[pp_][32:34, :]
                        nc.tensor.matmul(dst, lhsT, rhs,
                                         start=False, stop=False)
                    pending_m = []
                    if trace_on:
                        sq1 = [wp.tile([128, WCH], edt, tag="sq1",
                                       name=f"sq1_{p}_{e}") for p in P]
                        for p in P:
                            nc.vector.tensor_tensor(
                                out=sq1[p][:], in0=h1[p][:], in1=h1[p][:],
                                op=mul)
                        pu = [pp.tile([128, WCH], f32, tag="pu", bufs=2,
                                      name=f"pu_{p}_{e}") for p in P]
                        for p in P:
                            nc.tensor.matmul(pu[p][:], c_cblk, sq1[p][:],
                                             start=True, stop=True)
                    h2 = [wp.tile([128, WCH], edt, tag="h2", bufs=14,
                                  name=f"h2_{p}_{e}") for p in P]
                    for p in P:
                        nc.scalar.activation(h2[p][:], z2[p][:], Tanh,
                                             bias=c_b2[:, 0:1])
                    h2s[s] = h2
                    if s in c_w3y:
                        for p in P:
                            pending_m.append((c_w3y[s], h2[p][:],
                                              p, "full"))
                    # -- off-path trace branch --
                    if trace_on:
                        sq2 = [wp.tile([128, WCH], edt, tag="sq2",
                                       name=f"sq2_{p}_{e}") for p in P]
                        for p in P:
                            nc.gpsimd.tensor_tensor(
                                out=sq2[p][:], in0=h2[p][:], in1=h2[p][:],
                                op=mul)
                        w2t = [wp.tile([128, WCH], edt, tag="w2t",
                                       name=f"w2t_{p}_{e}") for p in P]
                        for p in P:
                            nc.vector.scalar_tensor_tensor(
                                out=w2t[p][:], in0=sq2[p][:], scalar=1.0,
                                in1=pu[p][:], op0=sub, op1=mul)
                        for p in P:
                            pending_m.append((c_m3b[s], sq2[p][:], p, "m"))
                            pending_m.append((c_m3c[s], w2t[p][:], p, "m"))
                # finish y update: flush remaining terms (per-pair stop)
                for p in P:
                    ent = [t for t in pending_m if t[2] == p]
                    ent.sort(key=lambda t: t[3] == "full")  # full-row last
                    for i, (lhsT, rhs, pp_, mode) in enumerate(ent):
                        dst = (yn[pp_][:] if mode == "full"
                               else yn[pp_][32:34, :])
                        nc.tensor.matmul(dst, lhsT, rhs, start=False,
                                         stop=(i == len(ent) - 1))
                pending_m = []
                for p in P:
                    st = sp.tile([SR, WCH], f32r, tag=f"y{p}",
                                 name=f"ynew{p}_{n}")
                    nc.scalar.copy(st[:], yn[p][:])
                    ystate[p] = st

            za = z_d.ap()
            lda = ld_d.ap()
            for p in P:
                for q in range(2):
                    c = 2 * p + q
                    nc.sync.dma_start(
                        out=za[:, c * WCH:(c + 1) * WCH],
                        in_=ystate[p][q * D:(q + 1) * D, :].bitcast(f32))
                nc.sync.dma_start(
                    out=lda[2 * p:2 * p + 2, :],
                    in_=ystate[p][32:34, :].bitcast(f32))

    nc.compile()
    return nc


_CACHE = {}


def _get_compiled(W1, b1, W2, b2, W3, b3):
    key = hash((W1.tobytes(), b1.tobytes(), W2.tobytes(), b2.tobytes(),
                W3.tobytes(), b3.tobytes()))
    if key not in _CACHE:
        consts = _host_consts(W1, b1, W2, b2, W3, b3)
        nc = _build(consts)
        _CACHE[key] = (nc, consts["K0"], consts["b3"].copy())
    return _CACHE[key]


LAST_RESULTS = None


def kernel(x, W1, b1, W2, b2, W3, b3):
    global LAST_RESULTS
    from concourse.bass_utils import run_bass_kernel_spmd

    x = np.asarray(x, np.float32)
    nc, K0, b3 = _get_compiled(
        np.asarray(W1, np.float32), np.asarray(b1, np.float32),
        np.asarray(W2, np.float32), np.asarray(b2, np.float32),
        np.asarray(W3, np.float32), np.asarray(b3, np.float32))

    in_maps = [{"x": np.ascontiguousarray(x[i * NPC:(i + 1) * NPC].T)}
               for i in range(NCORES)]
    trace = os.environ.get("CNF_TRACE", "0") == "1"
    res = run_bass_kernel_spmd(nc, in_maps, core_ids=list(range(NCORES)),
                               trace=trace)
    LAST_RESULTS = res
    z = np.concatenate([r["z"].T for r in res.results], axis=0) + b3[None, :]
    ld = np.concatenate([r["ld"].reshape(-1) for r in res.results],
                        axis=0) - np.float32(K0)
    return z.astype(np.float32), ld.astype(np.float32)


# revision 32
# speedup vs baseline: 1.0470x; 1.0470x over previous
"""Trainium2 Bass kernel for the CNF (continuous normalizing flow) problem.

Computes, for x ~ (16384, 8): 8 fixed dopri5 steps of the augmented ODE
  dx/dt = f(t, x) = tanh(tanh([x,t]W1+b1)W2+b2)W3 + b3
  dlogdet/dt = -tr(df/dx)
returning (z, delta_log_det).

Key reductions (host-precomputed constants):
  * tr(J) = d1^T C d2 with d_i = 1-h_i^2 and C[b,a] = W2[b,a]*(W3@W1[:8])[a,b].
    With s_i = h_i^2 and P = s1@C:  tr = K0 - c1.s2 - sum_a P_a(1-s2_a),
    needing only two squares, one extra matmul, and one fused (s2-1)*P op;
    all feature reductions ride inside matmul columns.
  * dopri5 stage combines x + dt*sum_j a_sj k_j fold into MM1's contraction:
    k_j tiles stack along partitions at 32-aligned offsets and MM1 adds one
    stacked pass with pre-scaled W1' blocks.  t/b3 feed-through terms fold
    into the per-eval tanh bias table.
  * the logdet pieces (m = c1.s2 - sum w2t per stage) accumulate directly in
    the y-update PSUM bank across the step with dt*b_j pre-scaled weights, so
    they never touch the critical path; stage 2's trace branch is skipped
    entirely (b[1] = 0).
  * state matmul passes run as float32r (1 cycle/row vs 4 for plain fp32).

Sharding: pure batch data-parallel over 8 cores (2048 samples/core).
Per-core layout: 4 chunks of 512 samples as 2 "pairs"; feature dim on
partitions (64 features x 2 chunks = 128 partitions).
"""

import os

import numpy as np

D = 8
H = 64
STEPS = 8
BATCH = 16384
NCORES = 8
NPC = BATCH // NCORES          # samples per core
WCH = 512                      # chunk width (columns per matmul stream)
NCHUNK = NPC // WCH            # 4
NPAIR = NCHUNK // 2            # 2
KR = 2 * D                     # 16 kx rows per pair
SR = 34                        # state tile rows: x at 0:16, ld at 32:34

DT = 1.0 / STEPS

# dopri5 tableau (matches reference._dopri5_step)
_A = [
    [],
    [0.2],
    [3 / 40, 9 / 40],
    [44 / 45, -56 / 15, 32 / 9],
    [19372 / 6561, -25360 / 2187, 64448 / 6561, -212 / 729],
    [9017 / 3168, -355 / 33, 46732 / 5247, 49 / 176, -5103 / 18656],
]
_B = [35 / 384, 0.0, 500 / 1113, 125 / 192, -2187 / 6784, 11 / 84]
_C = [0.0, 0.2, 0.3, 0.8, 8 / 9, 1.0]
NEVAL = 6 * STEPS

EVAL_BF16 = os.environ.get("CNF_FP32", "0") != "1"


def _host_consts(W1, b1, W2, b2, W3, b3):
    """All matmul weight blocks / bias tables, as fp32 numpy arrays."""
    f32 = np.float32
    W1 = np.asarray(W1, f32); b1 = np.asarray(b1, f32)
    W2 = np.asarray(W2, f32); b2 = np.asarray(b2, f32)
    W3 = np.asarray(W3, f32); b3 = np.asarray(b3, f32)
    W1p = W1[:D]                      # (8, 64)
    W1t = W1[D]                       # (64,)
    M = W3 @ W1p                      # (64, 64)
    C = W2 * M.T                      # C[b,a] = W2[b,a] * M[a,b]
    c1 = C.sum(axis=0)                # (64,)
    K0 = float(C.sum())
    b3W1 = b3 @ W1p                   # (64,)

    # per-eval tanh-1 bias: b1 + t*W1[8,:] + dt*(n + C_s)*(b3@W1')
    bias1 = np.zeros((128, NEVAL), f32)
    for n in range(STEPS):
        for s in range(6):
            e = 6 * n + s
            te = (n + _C[s]) * DT
            v = b1 + te * W1t + te * b3W1
            bias1[:H, e] = v
            bias1[H:, e] = v
    bias2 = np.concatenate([b2, b2]).reshape(128, 1).astype(f32)

    def blk(coef):
        """16-row block of scaled W1' for one k_j group of an MM1 lhsT."""
        b = np.zeros((32, 128), f32)
        b[0:D, 0:H] = coef * W1p
        b[D:2 * D, H:2 * H] = coef * W1p
        return b

    w1s = np.zeros((SR, 128), f32)    # state pass lhsT (34, 128)
    w1s[:KR] = blk(1.0)[:KR]

    # k_j groups: j=0..3 in kxgA at rows 32j, j=4 in kxgB at row 0
    def kstack(coefs, nrows):
        m = np.zeros((nrows, 128), f32)
        for j, cf in coefs:
            m[32 * j:32 * j + KR] = blk(cf)[:KR]
        return m

    # h2-direct: z1 contribution of k_j rides h2_j @ (coef * W31) with
    # W31 = W3 @ W1p, so k tiles are never materialized.
    W31 = W3 @ W1p                    # (64, 64)

    def w31blk(coef):
        b = np.zeros((128, 128), f32)
        b[0:H, 0:H] = coef * W31
        b[H:, H:] = coef * W31
        return b

    w31g = {}                         # stage s -> lhsT for the newest k
    for s in range(1, 6):
        w31g[s] = w31blk(DT * _A[s][s - 1])

    # stacked kx passes for k_0..k_{s-2} (kx tiles at 32-aligned rows)
    mm1g = {}                         # stage s>=2 -> lhsT over kxgA
    for s in range(2, 6):
        mm1g[s] = kstack([(j, DT * _A[s][j]) for j in range(s - 1)],
                         32 * (s - 2) + KR)

    m3a = np.zeros((128, KR), f32)    # kx = h2 @ W3
    m3a[0:H, 0:D] = W3
    m3a[H:, D:2 * D] = W3

    yug = np.zeros((112, SR), f32)    # y-update over kxgA (j=0..3)
    for j in range(4):
        for r in range(KR):
            yug[32 * j + r, r] = DT * _B[j]

    w2blk = np.zeros((128, 128), f32)
    w2blk[0:H, 0:H] = W2
    w2blk[H:, H:] = W2
    cblk = np.zeros((128, 128), f32)
    cblk[0:H, 0:H] = C
    cblk[H:, H:] = C


    # m-accumulators, pre-scaled by dt*b_s, writing yn rows 16:18
    m3b = []
    m3c = []
    for s in range(6):
        bpos = np.zeros((128, 2), f32)
        bpos[0:H, 0] = DT * _B[s] * c1
        bpos[H:, 1] = DT * _B[s] * c1
        m3b.append(bpos)
        bneg = np.zeros((128, 2), f32)
        bneg[0:H, 0] = -DT * _B[s]
        bneg[H:, 1] = -DT * _B[s]
        m3c.append(bneg)

    yupd_s = np.zeros((SR, SR), f32)  # carries x (0:16) and ld (32:34)
    for r in list(range(KR)) + [32, 33]:
        yupd_s[r, r] = 1.0

    w3y = {}                          # j -> y-update pass lhsT over h2_j
    for j in (4, 5):
        m = np.zeros((128, SR), f32)
        m[0:H, 0:D] = DT * _B[j] * W3
        m[H:, D:2 * D] = DT * _B[j] * W3
        w3y[j] = m

    return dict(w1s=w1s, w31g=w31g, mm1g=mm1g, m3a=m3a, yug=yug,
                cblk=cblk, m3b=m3b, m3c=m3c, yupd_s=yupd_s, w3y=w3y,
                w2blk=w2blk, bias1=bias1, bias2=bias2, K0=K0, b3=b3)


def _build(consts):
    """Build + compile the Bass module."""
    import ml_dtypes
    import concourse.bacc as bacc
    import concourse.mybir as mybir
    from concourse.tile import TileContext

    f32 = mybir.dt.float32
    f32r = mybir.dt.float32r
    edt = mybir.dt.bfloat16 if EVAL_BF16 else mybir.dt.float32
    enp = ml_dtypes.bfloat16 if EVAL_BF16 else np.float32

    nc = bacc.Bacc("TRN2", target_bir_lowering=False, debug=False)

    # x/z are host-transposed to feature-major for contiguous DMA
    x_d = nc.dram_tensor("x", [D, NPC], f32, kind="ExternalInput")
    z_d = nc.dram_tensor("z", [D, NPC], f32, kind="ExternalOutput")
    ld_d = nc.dram_tensor("ld", [NCHUNK, WCH], f32, kind="ExternalOutput")

    def inl(name, arr, npdt):
        return nc.inline_tensor(np.ascontiguousarray(arr.astype(npdt)),
                                name=name)

    # pack all lhsT/bias constants into two DRAM blocks (one DMA each)
    epack = []          # (name, array) -> column offsets, bf16 block
    fpack = []          # fp32 block
    def eadd(name, arr):
        a = np.zeros((128, arr.shape[1]), np.float32)
        a[:arr.shape[0]] = arr
        epack.append((name, a))
    def fadd(name, arr):
        a = np.zeros((128, arr.shape[1]), np.float32)
        a[:arr.shape[0]] = arr
        fpack.append((name, a))
    for s, v in consts["w31g"].items():
        eadd(f"w31g{s}", v)
    for s, v in consts["mm1g"].items():
        eadd(f"mm1g{s}", v)
    eadd("m3a", consts["m3a"])
    eadd("yug", consts["yug"])
    eadd("w2blk", consts["w2blk"])
    eadd("cblk", consts["cblk"])
    for s in range(6):
        eadd(f"m3b{s}", consts["m3b"][s])
        eadd(f"m3c{s}", consts["m3c"][s])
    for j, v in consts["w3y"].items():
        eadd(f"w3y{j}", v)
    fadd("w1s", consts["w1s"])
    fadd("yus", consts["yupd_s"])
    fadd("b1t", consts["bias1"])
    fadd("b2d", consts["bias2"])
    eoff = {}
    off = 0
    for name, a in epack:
        eoff[name] = (off, a.shape[1])
        off += a.shape[1]
    eblob = np.concatenate([a for _, a in epack], axis=1)
    foff = {}
    off = 0
    for name, a in fpack:
        foff[name] = (off, a.shape[1])
        off += a.shape[1]
    fblob = np.concatenate([a for _, a in fpack], axis=1)
    d_eblob = inl("eblob", eblob, enp)
    d_fblob = inl("fblob", fblob, np.float32)

    Tanh = mybir.ActivationFunctionType.Tanh
    Square = mybir.ActivationFunctionType.Square
    sub = mybir.AluOpType.subtract
    mul = mybir.AluOpType.mult

    with TileContext(nc) as tc:
        with tc.tile_pool(name="const", bufs=1) as cp, \
             tc.tile_pool(name="state", bufs=2) as sp, \
             tc.tile_pool(name="kxg", bufs=1) as kp, \
             tc.tile_pool(name="work", bufs=3) as wp, \
             tc.tile_pool(name="ps", bufs=2, space="PSUM") as pp, \
             tc.tile_pool(name="psg", bufs=1, space="PSUM") as pg:

            # state tiles first so their DMAs lead the queue.
            # state is f32r (1 cycle/row matmuls); producers must round.
            ystate = []
            xa = x_d.ap()
            for p in range(NPAIR):
                xs = sp.tile([SR, WCH], f32, tag=f"xs{p}")
                nc.vector.memset(xs[:], 0.0)
                for q in range(2):
                    c = 2 * p + q
                    nc.sync.dma_start(
                        out=xs[q * D:(q + 1) * D, :],
                        in_=xa[:, c * WCH:(c + 1) * WCH])
                st = sp.tile([SR, WCH], f32r, tag=f"y{p}")
                nc.vector.tensor_copy(out=st[:], in_=xs[:])
                ystate.append(st)

            ce = cp.tile([128, sum(w.shape[1] for _, w in epack)], edt,
                         name="c_eblob")
            nc.sync.dma_start(out=ce[:], in_=d_eblob.ap())
            cf = cp.tile([128, sum(w.shape[1] for _, w in fpack)], f32,
                         name="c_fblob")
            nc.sync.dma_start(out=cf[:], in_=d_fblob.ap())

            def esl(name, rows=128):
                o, w = eoff[name]
                return ce[0:rows, o:o + w]
            def fsl(name, rows=128):
                o, w = foff[name]
                return cf[0:rows, o:o + w]

            c_w31g = {s: esl(f"w31g{s}") for s in consts["w31g"]}
            c_mm1g = {s: esl(f"mm1g{s}", consts["mm1g"][s].shape[0])
                      for s in consts["mm1g"]}
            c_m3a = esl("m3a")
            c_yug = esl("yug", 112)
            c_w2blk = esl("w2blk")
            c_cblk = esl("cblk")
            c_m3b = [esl(f"m3b{s}") for s in range(6)]
            c_m3c = [esl(f"m3c{s}") for s in range(6)]
            c_w3y = {j: esl(f"w3y{j}") for j in consts["w3y"]}
            crs = cp.tile([SR, 2 * SR + 128], f32r, name="c_rblob")
            nc.vector.tensor_copy(
                out=crs[:, 0:128], in_=fsl("w1s", SR))
            nc.vector.tensor_copy(
                out=crs[:, 128:128 + SR], in_=fsl("yus", SR))
            c_w1s = crs[:, 0:128]
            c_yus = crs[:, 128:128 + SR]
            c_b1t = fsl("b1t")
            c_b2 = fsl("b2d")

            # persistent kx stacks: kx_j (16 rows) at rows 32j, j=0..3
            kxgA = []
            for p in range(NPAIR):
                ka = kp.tile([112, WCH], edt, tag=f"kA{p}", name=f"kxgA{p}")
                nc.vector.memset(ka[:], 0.0)
                kxgA.append(ka)

            P = range(NPAIR)
            for n in range(STEPS):
                # y-update accumulator: state carried over via identity
                # (start=True clears the bank), per-stage m-terms accumulate
                # into rows 32:34, dt*b_j k-combination lands at step end
                yn = [pg.tile([SR, WCH], f32, tag=f"yn{p}", name=f"yn{p}_{n}")
                      for p in P]
                for p in P:
                    nc.tensor.matmul(yn[p][:], c_yus,
                                     ystate[p][:],
                                     start=True, stop=False)
                pending_m = []      # deferred yn-accum matmuls
                h2s = {}            # stage index -> per-pair h2 tiles
                for s in range(6):
                    e = 6 * n + s
                    trace_on = _B[s] != 0.0
                    # -- critical chain: z1 -> tanh1 -> z2 -> tanh2 -> kx --
                    z1 = [pp.tile([128, WCH], f32, tag="zz", bufs=4,
                                  name=f"z1_{p}_{e}") for p in P]
                    for p in P:
                        nc.tensor.matmul(z1[p][:], c_w1s,
                                         ystate[p][:],
                                         start=True, stop=(s == 0))
                    if s >= 2:
                        for p in P:
                            nc.tensor.matmul(
                                z1[p][:], c_mm1g[s],
                                kxgA[p][0:32 * (s - 2) + KR, :],
                                start=False, stop=False)
                    if s >= 1:
                        for p in P:
                            nc.tensor.matmul(
                                z1[p][:], c_w31g[s], h2s[s - 1][p][:],
                                start=False, stop=True)
                    h1 = [wp.tile([128, WCH], edt, tag="h1",
                                  name=f"h1_{p}_{e}") for p in P]
                    for p in P:
                        nc.scalar.activation(h1[p][:], z1[p][:], Tanh,
                                             bias=c_b1t[:, e:e + 1])
                    z2 = [pp.tile([128, WCH], f32, tag="zz", bufs=4,
                                  name=f"z2_{p}_{e}") for p in P]
                    for p in P:
                        nc.tensor.matmul(z2[p][:], c_w2blk, h1[p][:],
                                         start=True, stop=True)
                    # deferred yn-accums from the previous stage fill PE's
                    # wait for tanh2
                    for lhsT, rhs, pp_, mode in pending_m:
                        dst = yn[pp_][:] if mode == "full" else yn[pp_][32:34, :]
                        nc.tensor.matmul(dst, lhsT, rhs,
                                         start=False, stop=False)
                    pending_m = []
                    if 1 <= s <= 4:
                        gf = [pg.tile([KR, WCH], f32, tag="gf", bufs=2,
                                      name=f"gf_{p}_{e}") for p in P]
                        for p in P:
                            nc.tensor.matmul(gf[p][:], c_m3a,
                                             h2s[s - 1][p][:],
                                             start=True, stop=True)
                        for p in P:
                            nc.vector.tensor_copy(
                                out=kxgA[p][32 * (s - 1):32 * (s - 1) + KR, :],
                                in_=gf[p][:])
                    if trace_on:
                        sq1 = [wp.tile([128, WCH], edt, tag="sq1",
                                       name=f"sq1_{p}_{e}") for p in P]
                        for p in P:
                            nc.vector.tensor_tensor(
                                out=sq1[p][:], in0=h1[p][:], in1=h1[p][:],
                                op=mul)
                        pu = [pp.tile([128, WCH], f32, tag="pu", bufs=2,
                                      name=f"pu_{p}_{e}") for p in P]
                        for p in P:
                            nc.tensor.matmul(pu[p][:], c_cblk, sq1[p][:],
                                             start=True, stop=True)
                    h2 = [wp.tile([128, WCH], edt, tag="h2", bufs=14,
                                  name=f"h2_{p}_{e}") for p in P]
                    for p in P:
                        nc.scalar.activation(h2[p][:], z2[p][:], Tanh,
                                             bias=c_b2[:, 0:1])
                    h2s[s] = h2
                    if s in c_w3y:
                        for p in P:
                            pending_m.append((c_w3y[s], h2[p][:],
                                              p, "full"))
                    # -- off-path trace branch --
                    if trace_on:
                        sq2 = [wp.tile([128, WCH], edt, tag="sq2",
                                       name=f"sq2_{p}_{e}") for p in P]
                        for p in P:
                            nc.gpsimd.tensor_tensor(
                                out=sq2[p][:], in0=h2[p][:], in1=h2[p][:],
                                op=mul)
                        w2t = [wp.tile([128, WCH], edt, tag="w2t",
                                       name=f"w2t_{p}_{e}") for p in P]
                        for p in P:
                            nc.vector.scalar_tensor_tensor(
                                out=w2t[p][:], in0=sq2[p][:], scalar=1.0,
                                in1=pu[p][:], op0=sub, op1=mul)
                        for p in P:
                            pending_m.append((c_m3b[s], sq2[p][:], p, "m"))
                            pending_m.append((c_m3c[s], w2t[p][:], p, "m"))
                # finish y update: stacked kx pass + remaining terms
                for p in P:
                    pending_m.append((c_yug, kxgA[p][:], p, "full"))
                for p in P:
                    ent = [t for t in pending_m if t[2] == p]
                    ent.sort(key=lambda t: t[3] == "full")  # full-row last
                    for i, (lhsT, rhs, pp_, mode) in enumerate(ent):
                        dst = (yn[pp_][:] if mode == "full"
                               else yn[pp_][32:34, :])
                        nc.tensor.matmul(dst, lhsT, rhs, start=False,
                                         stop=(i == len(ent) - 1))
                pending_m = []
                for p in P:
                    st = sp.tile([SR, WCH], f32r, tag=f"y{p}",
                                 name=f"ynew{p}_{n}")
                    nc.scalar.copy(st[:], yn[p][:])
                    ystate[p] = st

            za = z_d.ap()
            lda = ld_d.ap()
            for p in P:
                for q in range(2):
                    c = 2 * p + q
                    nc.sync.dma_start(
                        out=za[:, c * WCH:(c + 1) * WCH],
                        in_=ystate[p][q * D:(q + 1) * D, :].bitcast(f32))
                nc.sync.dma_start(
                    out=lda[2 * p:2 * p + 2, :],
                    in_=ystate[p][32:34, :].bitcast(f32))

    nc.compile()
    return nc


_CACHE = {}


def _get_compiled(W1, b1, W2, b2, W3, b3):
    key = hash((W1.tobytes(), b1.tobytes(), W2.tobytes(), b2.tobytes(),
                W3.tobytes(), b3.tobytes()))
    if key not in _CACHE:
        consts = _host_consts(W1, b1, W2, b2, W3, b3)
        nc = _build(consts)
        _CACHE[key] = (nc, consts["K0"], consts["b3"].copy())
    return _CACHE[key]


LAST_RESULTS = None


def kernel(x, W1, b1, W2, b2, W3, b3):
    global LAST_RESULTS
    from concourse.bass_utils import run_bass_kernel_spmd

    x = np.asarray(x, np.float32)
    nc, K0, b3 = _get_compiled(
        np.asarray(W1, np.float32), np.asarray(b1, np.float32),
        np.asarray(W2, np.float32), np.asarray(b2, np.float32),
        np.asarray(W3, np.float32), np.asarray(b3, np.float32))

    in_maps = [{"x": np.ascontiguousarray(x[i * NPC:(i + 1) * NPC].T)}
               for i in range(NCORES)]
    trace = os.environ.get("CNF_TRACE", "0") == "1"
    res = run_bass_kernel_spmd(nc, in_maps, core_ids=list(range(NCORES)),
                               trace=trace)
    LAST_RESULTS = res
    z = np.concatenate([r["z"].T for r in res.results], axis=0) + b3[None, :]
    ld = np.concatenate([r["ld"].reshape(-1) for r in res.results],
                        axis=0) - np.float32(K0)
    return z.astype(np.float32), ld.astype(np.float32)


# revision 36
# speedup vs baseline: 1.0671x; 1.0192x over previous
"""Trainium2 Bass kernel for the CNF (continuous normalizing flow) problem.

Computes, for x ~ (16384, 8): 8 fixed dopri5 steps of the augmented ODE
  dx/dt = f(t, x) = tanh(tanh([x,t]W1+b1)W2+b2)W3 + b3
  dlogdet/dt = -tr(df/dx)
returning (z, delta_log_det).

Key reductions (host-precomputed constants):
  * tr(J) = d1^T C d2 with d_i = 1-h_i^2 and C[b,a] = W2[b,a]*(W3@W1[:8])[a,b].
    With s_i = h_i^2 and P = s1@C:  tr = K0 - c1.s2 - sum_a P_a(1-s2_a),
    needing only two squares, one extra matmul, and one fused (s2-1)*P op;
    all feature reductions ride inside matmul columns.
  * dopri5 stage combines x + dt*sum_j a_sj k_j fold into MM1's contraction:
    k_j tiles stack along partitions at 32-aligned offsets and MM1 adds one
    stacked pass with pre-scaled W1' blocks.  t/b3 feed-through terms fold
    into the per-eval tanh bias table.
  * the logdet pieces (m = c1.s2 - sum w2t per stage) accumulate directly in
    the y-update PSUM bank across the step with dt*b_j pre-scaled weights, so
    they never touch the critical path; stage 2's trace branch is skipped
    entirely (b[1] = 0).
  * state matmul passes run as float32r (1 cycle/row vs 4 for plain fp32).

Sharding: pure batch data-parallel over 8 cores (2048 samples/core).
Per-core layout: 4 chunks of 512 samples as 2 "pairs"; feature dim on
partitions (64 features x 2 chunks = 128 partitions).
"""

import os

import numpy as np

D = 8
H = 64
STEPS = 8
BATCH = 16384
NCORES = 8
NPC = BATCH // NCORES          # samples per core
WCH = 512                      # chunk width (columns per matmul stream)
NCHUNK = NPC // WCH            # 4
NPAIR = NCHUNK // 2            # 2
KR = 2 * D                     # 16 kx rows per pair
SR = 34                        # state tile rows: x at 0:16, ld at 32:34

DT = 1.0 / STEPS

# dopri5 tableau (matches reference._dopri5_step)
_A = [
    [],
    [0.2],
    [3 / 40, 9 / 40],
    [44 / 45, -56 / 15, 32 / 9],
    [19372 / 6561, -25360 / 2187, 64448 / 6561, -212 / 729],
    [9017 / 3168, -355 / 33, 46732 / 5247, 49 / 176, -5103 / 18656],
]
_B = [35 / 384, 0.0, 500 / 1113, 125 / 192, -2187 / 6784, 11 / 84]
_C = [0.0, 0.2, 0.3, 0.8, 8 / 9, 1.0]
NEVAL = 6 * STEPS

EVAL_BF16 = os.environ.get("CNF_FP32", "0") != "1"


def _host_consts(W1, b1, W2, b2, W3, b3):
    """All matmul weight blocks / bias tables, as fp32 numpy arrays."""
    f32 = np.float32
    W1 = np.asarray(W1, f32); b1 = np.asarray(b1, f32)
    W2 = np.asarray(W2, f32); b2 = np.asarray(b2, f32)
    W3 = np.asarray(W3, f32); b3 = np.asarray(b3, f32)
    W1p = W1[:D]                      # (8, 64)
    W1t = W1[D]                       # (64,)
    M = W3 @ W1p                      # (64, 64)
    C = W2 * M.T                      # C[b,a] = W2[b,a] * M[a,b]
    c1 = C.sum(axis=0)                # (64,)
    K0 = float(C.sum())
    b3W1 = b3 @ W1p                   # (64,)

    # per-eval tanh-1 bias: b1 + t*W1[8,:] + dt*(n + C_s)*(b3@W1')
    bias1 = np.zeros((128, NEVAL), f32)
    for n in range(STEPS):
        for s in range(6):
            e = 6 * n + s
            te = (n + _C[s]) * DT
            v = b1 + te * W1t + te * b3W1
            bias1[:H, e] = v
            bias1[H:, e] = v
    bias2 = np.concatenate([b2, b2]).reshape(128, 1).astype(f32)

    def blk(coef):
        """16-row block of scaled W1' for one k_j group of an MM1 lhsT."""
        b = np.zeros((32, 128), f32)
        b[0:D, 0:H] = coef * W1p
        b[D:2 * D, H:2 * H] = coef * W1p
        return b

    w1s = np.zeros((SR, 128), f32)    # state pass lhsT (34, 128)
    w1s[:KR] = blk(1.0)[:KR]

    # k_j groups: j=0..3 in kxgA at rows 32j, j=4 in kxgB at row 0
    def kstack(coefs, nrows):
        m = np.zeros((nrows, 128), f32)
        for j, cf in coefs:
            m[32 * j:32 * j + KR] = blk(cf)[:KR]
        return m

    # h2-direct: z1 contribution of k_j rides h2_j @ (coef * W31) with
    # W31 = W3 @ W1p, so k tiles are never materialized.
    W31 = W3 @ W1p                    # (64, 64)

    def w31blk(coef):
        b = np.zeros((128, 128), f32)
        b[0:H, 0:H] = coef * W31
        b[H:, H:] = coef * W31
        return b

    w31g = {}                         # stage s -> lhsT for the newest k
    for s in range(1, 6):
        w31g[s] = w31blk(DT * _A[s][s - 1])

    # delta-coefficient stacked passes: z1 accumulates in PSUM across
    # stages, so stage s only adds dt*(a_sj - a_{s-1,j}) * k_j for the
    # already-present k's (telescopes to exact a_sj) plus the newest k.
    mm1g = {}                         # stage s>=2 -> delta lhsT over kxgA
    for s in range(2, 6):
        mm1g[s] = kstack(
            [(j, DT * (_A[s][j] - _A[s - 1][j])) for j in range(s - 1)],
            32 * (s - 2) + KR)

    m3a = np.zeros((128, KR), f32)    # kx = h2 @ W3
    m3a[0:H, 0:D] = W3
    m3a[H:, D:2 * D] = W3

    yug = np.zeros((112, SR), f32)    # y-update over kxgA (j=0..3)
    for j in range(4):
        for r in range(KR):
            yug[32 * j + r, r] = DT * _B[j]

    w2blk = np.zeros((128, 128), f32)
    w2blk[0:H, 0:H] = W2
    w2blk[H:, H:] = W2
    cblk = np.zeros((128, 128), f32)
    cblk[0:H, 0:H] = C
    cblk[H:, H:] = C


    # m-accumulators, pre-scaled by dt*b_s, writing yn rows 16:18
    m3b = []
    m3c = []
    for s in range(6):
        bpos = np.zeros((128, 2), f32)
        bpos[0:H, 0] = DT * _B[s] * c1
        bpos[H:, 1] = DT * _B[s] * c1
        m3b.append(bpos)
        bneg = np.zeros((128, 2), f32)
        bneg[0:H, 0] = -DT * _B[s]
        bneg[H:, 1] = -DT * _B[s]
        m3c.append(bneg)

    yupd_s = np.zeros((SR, SR), f32)  # carries x (0:16) and ld (32:34)
    for r in list(range(KR)) + [32, 33]:
        yupd_s[r, r] = 1.0

    w3y = {}                          # j -> y-update pass lhsT over h2_j
    for j in (4, 5):
        m = np.zeros((128, SR), f32)
        m[0:H, 0:D] = DT * _B[j] * W3
        m[H:, D:2 * D] = DT * _B[j] * W3
        w3y[j] = m

    return dict(w1s=w1s, w31g=w31g, mm1g=mm1g, m3a=m3a, yug=yug,
                cblk=cblk, m3b=m3b, m3c=m3c, yupd_s=yupd_s, w3y=w3y,
                w2blk=w2blk, bias1=bias1, bias2=bias2, K0=K0, b3=b3)


def _build(consts):
    """Build + compile the Bass module."""
    import ml_dtypes
    import concourse.bacc as bacc
    import concourse.mybir as mybir
    from concourse.tile import TileContext

    f32 = mybir.dt.float32
    f32r = mybir.dt.float32r
    edt = mybir.dt.bfloat16 if EVAL_BF16 else mybir.dt.float32
    enp = ml_dtypes.bfloat16 if EVAL_BF16 else np.float32

    nc = bacc.Bacc("TRN2", target_bir_lowering=False, debug=False)

    # x/z are host-transposed to feature-major for contiguous DMA
    x_d = nc.dram_tensor("x", [D, NPC], f32, kind="ExternalInput")
    z_d = nc.dram_tensor("z", [D, NPC], f32, kind="ExternalOutput")
    ld_d = nc.dram_tensor("ld", [NCHUNK, WCH], f32, kind="ExternalOutput")

    def inl(name, arr, npdt):
        return nc.inline_tensor(np.ascontiguousarray(arr.astype(npdt)),
                                name=name)

    # pack all lhsT/bias constants into two DRAM blocks (one DMA each)
    epack = []          # (name, array) -> column offsets, bf16 block
    fpack = []          # fp32 block
    def eadd(name, arr):
        a = np.zeros((128, arr.shape[1]), np.float32)
        a[:arr.shape[0]] = arr
        epack.append((name, a))
    def fadd(name, arr):
        a = np.zeros((128, arr.shape[1]), np.float32)
        a[:arr.shape[0]] = arr
        fpack.append((name, a))
    for s, v in consts["w31g"].items():
        eadd(f"w31g{s}", v)
    for s, v in consts["mm1g"].items():
        eadd(f"mm1g{s}", v)
    eadd("m3a", consts["m3a"])
    eadd("yug", consts["yug"])
    eadd("w2blk", consts["w2blk"])
    eadd("cblk", consts["cblk"])
    for s in range(6):
        eadd(f"m3b{s}", consts["m3b"][s])
        eadd(f"m3c{s}", consts["m3c"][s])
    for j, v in consts["w3y"].items():
        eadd(f"w3y{j}", v)
    fadd("w1s", consts["w1s"])
    fadd("yus", consts["yupd_s"])
    fadd("b1t", consts["bias1"])
    fadd("b2d", consts["bias2"])
    eoff = {}
    off = 0
    for name, a in epack:
        eoff[name] = (off, a.shape[1])
        off += a.shape[1]
    eblob = np.concatenate([a for _, a in epack], axis=1)
    foff = {}
    off = 0
    for name, a in fpack:
        foff[name] = (off, a.shape[1])
        off += a.shape[1]
    fblob = np.concatenate([a for _, a in fpack], axis=1)
    d_eblob = inl("eblob", eblob, enp)
    d_fblob = inl("fblob", fblob, np.float32)

    Tanh = mybir.ActivationFunctionType.Tanh
    Square = mybir.ActivationFunctionType.Square
    sub = mybir.AluOpType.subtract
    mul = mybir.AluOpType.mult

    with TileContext(nc) as tc:
        with tc.tile_pool(name="const", bufs=1) as cp, \
             tc.tile_pool(name="state", bufs=2) as sp, \
             tc.tile_pool(name="kxg", bufs=1) as kp, \
             tc.tile_pool(name="work", bufs=3) as wp, \
             tc.tile_pool(name="ps", bufs=2, space="PSUM") as pp, \
             tc.tile_pool(name="psg", bufs=1, space="PSUM") as pg:

            # state tiles first so their DMAs lead the queue.
            # state is f32r (1 cycle/row matmuls); producers must round.
            ystate = []
            xa = x_d.ap()
            for p in range(NPAIR):
                xs = sp.tile([SR, WCH], f32, tag=f"xs{p}")
                nc.vector.memset(xs[:], 0.0)
                for q in range(2):
                    c = 2 * p + q
                    nc.sync.dma_start(
                        out=xs[q * D:(q + 1) * D, :],
                        in_=xa[:, c * WCH:(c + 1) * WCH])
                st = sp.tile([SR, WCH], f32r, tag=f"y{p}")
                nc.vector.tensor_copy(out=st[:], in_=xs[:])
                ystate.append(st)

            ce = cp.tile([128, sum(w.shape[1] for _, w in epack)], edt,
                         name="c_eblob")
            nc.sync.dma_start(out=ce[:], in_=d_eblob.ap())
            cf = cp.tile([128, sum(w.shape[1] for _, w in fpack)], f32,
                         name="c_fblob")
            nc.sync.dma_start(out=cf[:], in_=d_fblob.ap())

            def esl(name, rows=128):
                o, w = eoff[name]
                return ce[0:rows, o:o + w]
            def fsl(name, rows=128):
                o, w = foff[name]
                return cf[0:rows, o:o + w]

            c_w31g = {s: esl(f"w31g{s}") for s in consts["w31g"]}
            c_mm1g = {s: esl(f"mm1g{s}", consts["mm1g"][s].shape[0])
                      for s in consts["mm1g"]}
            c_m3a = esl("m3a")
            c_yug = esl("yug", 112)
            c_w2blk = esl("w2blk")
            c_cblk = esl("cblk")
            c_m3b = [esl(f"m3b{s}") for s in range(6)]
            c_m3c = [esl(f"m3c{s}") for s in range(6)]
            c_w3y = {j: esl(f"w3y{j}") for j in consts["w3y"]}
            crs = cp.tile([SR, 2 * SR + 128], f32r, name="c_rblob")
            nc.vector.tensor_copy(
                out=crs[:, 0:128], in_=fsl("w1s", SR))
            nc.vector.tensor_copy(
                out=crs[:, 128:128 + SR], in_=fsl("yus", SR))
            c_w1s = crs[:, 0:128]
            c_yus = crs[:, 128:128 + SR]
            c_b1t = fsl("b1t")
            c_b2 = fsl("b2d")

            # persistent kx stacks: kx_j (16 rows) at rows 32j, j=0..3
            kxgA = []
            for p in range(NPAIR):
                ka = kp.tile([112, WCH], edt, tag=f"kA{p}", name=f"kxgA{p}")
                nc.vector.memset(ka[:], 0.0)
                kxgA.append(ka)

            P = range(NPAIR)
            for n in range(STEPS):
                # y-update accumulator: state carried over via identity
                # (start=True clears the bank), per-stage m-terms accumulate
                # into rows 32:34, dt*b_j k-combination lands at step end
                yn = [pg.tile([SR, WCH], f32, tag=f"yn{p}", name=f"yn{p}_{n}")
                      for p in P]
                for p in P:
                    nc.tensor.matmul(yn[p][:], c_yus,
                                     ystate[p][:],
                                     start=True, stop=False)
                pending_m = []      # deferred yn-accum matmuls
                h2s = {}            # stage index -> per-pair h2 tiles
                for s in range(6):
                    e = 6 * n + s
                    trace_on = _B[s] != 0.0
                    # -- critical chain: z1 -> tanh1 -> z2 -> tanh2 -> kx --
                    z1 = [pp.tile([128, WCH], f32, tag="zz", bufs=4,
                                  name=f"z1_{p}_{e}") for p in P]
                    for p in P:
                        nc.tensor.matmul(z1[p][:], c_w1s,
                                         ystate[p][:],
                                         start=True, stop=(s == 0))
                    if s >= 2:
                        for p in P:
                            nc.tensor.matmul(
                                z1[p][:], c_mm1g[s],
                                kxgA[p][0:32 * (s - 2) + KR, :],
                                start=False, stop=False)
                    if s >= 1:
                        for p in P:
                            nc.tensor.matmul(
                                z1[p][:], c_w31g[s], h2s[s - 1][p][:],
                                start=False, stop=True)
                    h1 = [wp.tile([128, WCH], edt, tag="h1",
                                  name=f"h1_{p}_{e}") for p in P]
                    for p in P:
                        nc.scalar.activation(h1[p][:], z1[p][:], Tanh,
                                             bias=c_b1t[:, e:e + 1])
                    z2 = [pp.tile([128, WCH], f32, tag="zz", bufs=4,
                                  name=f"z2_{p}_{e}") for p in P]
                    for p in P:
                        nc.tensor.matmul(z2[p][:], c_w2blk, h1[p][:],
                                         start=True, stop=True)
                    if 1 <= s <= 4:
                        gf = [pg.tile([KR, WCH], f32, tag="gf", bufs=2,
                                      name=f"gf_{p}_{e}") for p in P]
                        for p in P:
                            nc.tensor.matmul(gf[p][:], c_m3a,
                                             h2s[s - 1][p][:],
                                             start=True, stop=True)
                        for p in P:
                            nc.vector.tensor_copy(
                                out=kxgA[p][32 * (s - 1):32 * (s - 1) + KR, :],
                                in_=gf[p][:])
                    # deferred yn-accums from the previous stage fill PE's
                    # wait for tanh2
                    for lhsT, rhs, pp_, mode in pending_m:
                        dst = yn[pp_][:] if mode == "full" else yn[pp_][32:34, :]
                        nc.tensor.matmul(dst, lhsT, rhs,
                                         start=False, stop=False)
                    pending_m = []
                    if trace_on:
                        sq1 = [wp.tile([128, WCH], edt, tag="sq1",
                                       name=f"sq1_{p}_{e}") for p in P]
                        for p in P:
                            nc.vector.tensor_tensor(
                                out=sq1[p][:], in0=h1[p][:], in1=h1[p][:],
                                op=mul)
                        pu = [pp.tile([128, WCH], f32, tag="pu", bufs=1,
                                      name=f"pu_{p}_{e}") for p in P]
                        for p in P:
                            nc.tensor.matmul(pu[p][:], c_cblk, sq1[p][:],
                                             start=True, stop=True)
                    h2 = [wp.tile([128, WCH], edt, tag="h2", bufs=14,
                                  name=f"h2_{p}_{e}") for p in P]
                    for p in P:
                        nc.scalar.activation(h2[p][:], z2[p][:], Tanh,
                                             bias=c_b2[:, 0:1])
                    h2s[s] = h2
                    if s in c_w3y:
                        for p in P:
                            pending_m.append((c_w3y[s], h2[p][:],
                                              p, "full"))
                    # -- off-path trace branch --
                    if trace_on:
                        sq2 = [wp.tile([128, WCH], edt, tag="sq2",
                                       name=f"sq2_{p}_{e}") for p in P]
                        for p in P:
                            nc.gpsimd.tensor_tensor(
                                out=sq2[p][:], in0=h2[p][:], in1=h2[p][:],
                                op=mul)
                        w2t = [wp.tile([128, WCH], edt, tag="w2t",
                                       name=f"w2t_{p}_{e}") for p in P]
                        for p in P:
                            nc.vector.scalar_tensor_tensor(
                                out=w2t[p][:], in0=sq2[p][:], scalar=1.0,
                                in1=pu[p][:], op0=sub, op1=mul)
                        for p in P:
                            pending_m.append((c_m3b[s], sq2[p][:], p, "m"))
                            pending_m.append((c_m3c[s], w2t[p][:], p, "m"))
                # finish y update: stacked kx pass + remaining terms
                for p in P:
                    pending_m.append((c_yug, kxgA[p][:], p, "full"))
                for p in P:
                    ent = [t for t in pending_m if t[2] == p]
                    ent.sort(key=lambda t: t[3] == "full")  # full-row last
                    for i, (lhsT, rhs, pp_, mode) in enumerate(ent):
                        dst = (yn[pp_][:] if mode == "full"
                               else yn[pp_][32:34, :])
                        nc.tensor.matmul(dst, lhsT, rhs, start=False,
                                         stop=(i == len(ent) - 1))
                pending_m = []
                for p in P:
                    st = sp.tile([SR, WCH], f32r, tag=f"y{p}",
                                 name=f"ynew{p}_{n}")
                    nc.scalar.copy(st[:], yn[p][:])
                    ystate[p] = st

            za = z_d.ap()
            lda = ld_d.ap()
            for p in P:
                for q in range(2):
                    c = 2 * p + q
                    nc.sync.dma_start(
                        out=za[:, c * WCH:(c + 1) * WCH],
                        in_=ystate[p][q * D:(q + 1) * D, :].bitcast(f32))
                nc.sync.dma_start(
                    out=lda[2 * p:2 * p + 2, :],
                    in_=ystate[p][32:34, :].bitcast(f32))

    nc.compile()
    return nc


_CACHE = {}


def _get_compiled(W1, b1, W2, b2, W3, b3):
    key = hash((W1.tobytes(), b1.tobytes(), W2.tobytes(), b2.tobytes(),
                W3.tobytes(), b3.tobytes()))
    if key not in _CACHE:
        consts = _host_consts(W1, b1, W2, b2, W3, b3)
        nc = _build(consts)
        _CACHE[key] = (nc, consts["K0"], consts["b3"].copy())
    return _CACHE[key]


LAST_RESULTS = None


def kernel(x, W1, b1, W2, b2, W3, b3):
    global LAST_RESULTS
    from concourse.bass_utils import run_bass_kernel_spmd

    x = np.asarray(x, np.float32)
    nc, K0, b3 = _get_compiled(
        np.asarray(W1, np.float32), np.asarray(b1, np.float32),
        np.asarray(W2, np.float32), np.asarray(b2, np.float32),
        np.asarray(W3, np.float32), np.asarray(b3, np.float32))

    in_maps = [{"x": np.ascontiguousarray(x[i * NPC:(i + 1) * NPC].T)}
               for i in range(NCORES)]
    trace = os.environ.get("CNF_TRACE", "0") == "1"
    res = run_bass_kernel_spmd(nc, in_maps, core_ids=list(range(NCORES)),
                               trace=trace)
    LAST_RESULTS = res
    z = np.concatenate([r["z"].T for r in res.results], axis=0) + b3[None, :]
    ld = np.concatenate([r["ld"].reshape(-1) for r in res.results],
                        axis=0) - np.float32(K0)
    return z.astype(np.float32), ld.astype(np.float32)
